# revision 18
# baseline (speedup 1.0000x reference)
"""Bahdanau 1D attention Trainium2 kernel.

Math (mask is all-ones per the problem spec, so it is algebraically dropped):
  values[t,u] = x[t,:] @ W            (computed transposed: [U, T] on chip)
  score[t]    = sum_u v_u * tanh(b_u + values[t,u])
  e[t]        = exp(score[t]);  S = sum_t e[t]
  weights[t]  = e[t] / S
  output[u]   = sum_t weights[t] * values[t,u]
              = ((sum_t e[t] * x[t,:]) @ W) / S        <- linear factorization:
  the weighted sum commutes with the matmul, so raw `values` in [T,U] layout
  are never needed - only tanh(values^T) for scores.

Sharding: data-parallel over batch, 8 batches per core on 8 cores.

Per-core loop (8 batches x 16 chunks of T=512):
  - DMA x in 2 MiB pieces  [128(T%128), 4096] natural layout
  - PE transposes x -> xT [D, T] (PSUM), DVE copies PSUM->SBUF
  - PE values^T = W^T @ xT (fp32r, full rate at N=512)
  - ACT tanh with per-partition bias b
  - PE score^T columns [128,1] via y-stationary matmuls against v
  - ACT exp -> e_all [128, 64] per batch
  - PE xw += e^T @ x   (accumulates [1, 256] in PSUM across the whole batch)
  - batch epilogue: S, 1/S, weights = e/S (transposed out), out = (xw@W)/S
"""

import sys

for _p in ("/opt/trn_rl_repo",):
    if _p not in sys.path:
        sys.path.insert(0, _p)

import numpy as np

B, T, D, U = 64, 8192, 256, 256
NCORES = 8
BPC = B // NCORES          # batches per core
PIECE = 2048               # T rows per DMA
CHUNK = 512                # T rows per compute chunk
NCHUNK = T // CHUNK        # 16
NPIECE = T // PIECE        # 4
SUBPC = PIECE // CHUNK     # 4

_CACHE = {}


def _build_program():
    import concourse.bass as bass  # noqa: F401
    import concourse.tile as tile
    from concourse import bacc, mybir
    from contextlib import ExitStack

    f32 = mybir.dt.float32
    f32r = mybir.dt.float32r
    bf16 = mybir.dt.bfloat16  # noqa: F841
    fp16 = mybir.dt.float16
    AF = mybir.ActivationFunctionType

    nc = bacc.Bacc(
        "TRN2", target_bir_lowering=False, debug=False, num_devices=NCORES
    )

    x_d = nc.dram_tensor("x", [BPC, T, D], fp16, kind="ExternalInput").ap()
    w_d = nc.dram_tensor("wmat", [D, U], f32r, kind="ExternalInput").ap()
    b_d = nc.dram_tensor("bvec", [128, 2], f32, kind="ExternalInput").ap()
    v_d = nc.dram_tensor("vvec", [128, 4], f32r, kind="ExternalInput").ap()
    id_d = nc.dram_tensor("ident", [128, 128], f32r, kind="ExternalInput").ap()
    onc_d = nc.dram_tensor("onesc", [128, 2], f32r, kind="ExternalInput").ap()
    onr_d = nc.dram_tensor("onesr", [1, 128], f32r, kind="ExternalInput").ap()
    out_d = nc.dram_tensor("out", [BPC, U], f32, kind="ExternalOutput").ap()
    wts_d = nc.dram_tensor("wts", [BPC, T], f32, kind="ExternalOutput").ap()

    def r(ap):  # fp32 -> fp32r view for full-rate PE matmuls
        return ap.bitcast(f32r)

    with tile.TileContext(nc) as tc, ExitStack() as ctx:
        const = ctx.enter_context(tc.tile_pool(name="const", bufs=1))
        xin = ctx.enter_context(tc.tile_pool(name="xin", bufs=3))
        xt_sb = ctx.enter_context(tc.tile_pool(name="xtsb", bufs=6))
        v_ps = ctx.enter_context(tc.tile_pool(name="vps", bufs=4, space="PSUM"))
        y_sb = ctx.enter_context(tc.tile_pool(name="ysb", bufs=6))
        s_ps = ctx.enter_context(tc.tile_pool(name="sps", bufs=1, space="PSUM"))
        e_sb = ctx.enter_context(tc.tile_pool(name="esb", bufs=2))
        e16_sb = ctx.enter_context(tc.tile_pool(name="e16sb", bufs=3))
        xw_ps = ctx.enter_context(tc.tile_pool(name="xwps", bufs=2, space="PSUM"))
        epi_ps = ctx.enter_context(tc.tile_pool(name="epips", bufs=1, space="PSUM"))
        epi_sb = ctx.enter_context(tc.tile_pool(name="episb", bufs=2))

        W_sb = const.tile([128, 512], f32r, tag="W")  # col h*256+u = W[h*128+d, u]
        nc.sync.dma_start(W_sb[:, 0:256], w_d[0:128, :])
        nc.sync.dma_start(W_sb[:, 256:512], w_d[128:256, :])
        b_sb = const.tile([128, 2], f32, tag="bv")
        nc.sync.dma_start(b_sb[:, :], b_d[:, :])
        v_sb = const.tile([128, 4], f32r, tag="vv")
        nc.sync.dma_start(v_sb[:, :], v_d[:, :])
        ident = const.tile([128, 128], f32r, tag="id")
        nc.sync.dma_start(ident[:, :], id_d[:, :])
        onesc = const.tile([128, 2], f32r, tag="onc")
        nc.sync.dma_start(onesc[:, :], onc_d[:, :])
        onesr = const.tile([1, 128], f32r, tag="onr")
        nc.sync.dma_start(onesr[:, :], onr_d[:, :])
        v_bf = const.tile([128, 4], fp16, tag="vbf")
        nc.vector.tensor_copy(v_bf[:, :], v_sb[:, :].bitcast(f32))
        W_h16 = const.tile([128, 512], fp16, tag="Wh")
        nc.vector.tensor_copy(W_h16[:, :], W_sb[:, :].bitcast(f32))
        ident_h = const.tile([128, 128], fp16, tag="idh")
        nc.vector.tensor_copy(ident_h[:, :], ident[:, :].bitcast(f32))

        batch_state = {}

        def emit_back(st):
            """score -> exp -> xw for a completed front-stage chunk."""
            bi, c, sc, xbig, ys = st
            if c == 0:
                e_all = e_sb.tile([128, NCHUNK * 4], f32r, tag="e",
                                  name=f"eall{bi}")
                xw = xw_ps.tile([1, 256], f32, tag="xw", name=f"xw{bi}")
                batch_state[bi] = (e_all, xw)
            e_all, xw = batch_state[bi]
            s = s_ps.tile([128, 8], f32, tag="s", name=f"s{bi}_{c}")
            for g in range(4):
                for m in range(2):
                    nc.tensor.matmul(
                        s[:, 2 * g:2 * g + 2],
                        ys[m][:, g * 128:(g + 1) * 128],
                        v_bf[:, 2 * m:2 * m + 2],
                        start=(m == 0), stop=(m == 1),
                        skip_group_check=True)
            s_even = s[:, :].rearrange("p (g two) -> p g two", two=2)[:, :, 0:1]
            e_dst = e_all[:, c * 4:(c + 1) * 4].rearrange(
                "p (g one) -> p g one", one=1)
            nc.scalar.activation(e_dst, s_even, AF.Exp)
            e16 = e16_sb.tile([128, 4], fp16, tag="e16", name=f"e16_{bi}_{c}")
            nc.vector.tensor_copy(
                e16[:, :], e_all[:, c * 4:(c + 1) * 4].bitcast(f32))
            for g in range(4):
                nc.tensor.matmul(
                    xw[:, :],
                    e16[:, g:g + 1],
                    xbig[:, (sc * 4 + g) * 256:(sc * 4 + g + 1) * 256],
                    start=(c == 0 and g == 0),
                    stop=(c == NCHUNK - 1 and g == 3),
                    skip_group_check=True)

        def emit_epilogue(bi):
            e_all, xw = batch_state.pop(bi)
            sum_col = epi_sb.tile([128, 1], f32, tag="sumcol",
                                  name=f"sumcol{bi}")
            nc.vector.tensor_reduce(
                sum_col[:, :], e_all[:, :],
                axis=mybir.AxisListType.X, op=mybir.AluOpType.add)
            sum_col_r = epi_sb.tile([128, 1], f32r, tag="sumcolr",
                                    name=f"sumcolr{bi}")
            nc.vector.tensor_copy(sum_col_r[:, :], sum_col[:, :])
            stot_ps = epi_ps.tile([1, 2], f32, tag="epiA", name=f"stot{bi}")
            nc.tensor.matmul(stot_ps[:, :], sum_col_r[:, :], onesc[:, :],
                             start=True, stop=True, skip_group_check=True)
            rcp = epi_sb.tile([1, 2], f32, tag="rcp", name=f"rcp{bi}")
            nc.vector.reciprocal(rcp[:, :], stot_ps[0:1, 0:2])
            rcp_r = epi_sb.tile([1, 2], f32r, tag="rcpr", name=f"rcpr{bi}")
            nc.vector.tensor_copy(rcp_r[:, :], rcp[:, :])
            rep_ps = epi_ps.tile([128, 2], f32, tag="epiA", name=f"rep{bi}")
            nc.tensor.matmul(rep_ps[:, :], onesr[:, :], rcp_r[:, :],
                             start=True, stop=True, skip_group_check=True)
            rcol = epi_sb.tile([128, 1], f32, tag="rcol", name=f"rcol{bi}")
            nc.vector.tensor_copy(rcol[:, :], rep_ps[:, 0:1])
            eT_ps = epi_ps.tile([64, 128], f32, tag="epiA", name=f"eT{bi}")
            nc.tensor.matmul(r(eT_ps[:, :]), r(e_all[:, :]), r(ident[:, :]),
                             is_transpose=True, start=True, stop=True,
                             skip_group_check=True)
            wT_sb = epi_sb.tile([64, 128], f32, tag="wT", name=f"wT{bi}")
            nc.scalar.activation(wT_sb[:, :], eT_ps[:, :], AF.Copy,
                                 scale=rcol[0:64, :])
            nc.sync.dma_start(
                wts_d[bi].rearrange("(c q) -> c q", q=128), wT_sb[:, :])
            xw_sb = epi_sb.tile([1, 256], f32r, tag="xwsb", name=f"xwsb{bi}")
            nc.vector.tensor_copy(xw_sb[:, :], xw[:, :])
            xwT_ps = epi_ps.tile([128, 4], f32, tag="epiA", name=f"xwT{bi}")
            for h in range(2):
                nc.tensor.matmul(
                    r(xwT_ps[:, 2 * h:2 * h + 2]),
                    r(xw_sb[:, h * 128:(h + 1) * 128]),
                    onesr[0:1, 0:2], is_transpose=True,
                    start=True, stop=True, skip_group_check=True)
            xwT_sb = epi_sb.tile([128, 4], f32r, tag="xwT", name=f"xwTs{bi}")
            nc.vector.tensor_copy(xwT_sb[:, :], r(xwT_ps[:, :]))
            out_ps = epi_ps.tile([1, 256], f32, tag="epiA", name=f"outps{bi}")
            for h in range(2):
                nc.tensor.matmul(out_ps[:, :], r(xwT_sb[:, 2 * h:2 * h + 1]),
                                 r(W_sb[:, h * 256:(h + 1) * 256]),
                                 start=(h == 0), stop=(h == 1),
                                 skip_group_check=True)
            out_sb = epi_sb.tile([1, 256], f32, tag="osb", name=f"osb{bi}")
            nc.scalar.activation(out_sb[:, :], out_ps[:, :], AF.Copy,
                                 scale=rcp[0:1, 0:1])
            nc.sync.dma_start(out_d[bi:bi + 1, :], out_sb[:, :])

        pend = None
        for bi in range(BPC):
            for piece in range(NPIECE):
                xbig = xin.tile([128, PIECE * D // 128], fp16, tag="x",
                                name=f"xbig{bi}_{piece}")
                src = x_d[bi, piece * PIECE:(piece + 1) * PIECE, :].rearrange(
                    "(k p) d -> p k d", p=128
                )
                dst = xbig[:, :].rearrange("p (k d) -> p k d", d=D)
                nc.sync.dma_start(dst, src)

                for sc in range(SUBPC):
                    c = piece * SUBPC + sc
                    t0 = piece * PIECE + sc * CHUNK
                    xts = [xt_sb.tile([128, 512], fp16, tag="xts",
                                      name=f"xts{_h}") for _h in range(2)]
                    for h in range(2):
                        nc.sync.dma_start_transpose(
                            xts[h][:, :],
                            x_d[bi, t0:t0 + CHUNK, h * 128:(h + 1) * 128])
                    vps = [v_ps.tile([128, 512], f32, tag="v",
                                     name=f"vps{_h}") for _h in range(2)]
                    for m in range(2):
                        for h in range(2):
                            nc.tensor.matmul(
                                vps[m][:, :],
                                W_h16[:, h * 256 + m * 128:h * 256 + (m + 1) * 128],
                                xts[h][:, :],
                                start=(h == 0), stop=(h == 1),
                                skip_group_check=True)
                    ys = [y_sb.tile([128, 512], fp16, tag="y",
                                    name=f"ys{_h}") for _h in range(2)]
                    for m in range(2):
                        nc.scalar.activation(
                            ys[m][:, :], vps[m][:, :], AF.Tanh,
                            bias=b_sb[:, m:m + 1], scale=1.0)
                    if pend is not None:
                        emit_back(pend)
                        if pend[1] == NCHUNK - 1:
                            emit_epilogue(pend[0])
                    pend = (bi, c, sc, xbig, ys)
        emit_back(pend)
        emit_epilogue(pend[0])

    nc.compile()
    return nc


def _get_program():
    if "nc" not in _CACHE:
        _CACHE["nc"] = _build_program()
    return _CACHE["nc"]


def _make_in_maps(x, W, b, v):
    b2 = np.ascontiguousarray(b.reshape(2, 128).T)
    v2c = v.reshape(2, 128).T
    v4 = np.empty((128, 4), dtype=np.float32)
    v4[:, 0] = v2c[:, 0]
    v4[:, 1] = v2c[:, 0]
    v4[:, 2] = v2c[:, 1]
    v4[:, 3] = v2c[:, 1]
    eye = np.eye(128, dtype=np.float32)
    onesc = np.ones((128, 2), dtype=np.float32)
    onesr = np.ones((1, 128), dtype=np.float32)
    x16 = np.asarray(x, dtype=np.float16)
    return [{
        "x": np.ascontiguousarray(x16[c * BPC:(c + 1) * BPC]),
        "wmat": W, "bvec": b2, "vvec": v4,
        "ident": eye, "onesc": onesc, "onesr": onesr,
    } for c in range(NCORES)]


def kernel(**inputs):
    from concourse.bass_utils import run_bass_kernel_spmd

    x = np.ascontiguousarray(np.asarray(inputs["inputs"], dtype=np.float32))
    W = np.ascontiguousarray(np.asarray(inputs["W"], dtype=np.float32))
    b = np.asarray(inputs["b"], dtype=np.float32)
    v = np.asarray(inputs["v"], dtype=np.float32)
    # mask is all-ones by problem construction; algebraically a no-op.

    nc = _get_program()
    in_maps = _make_in_maps(x, W, b, v)
    res = run_bass_kernel_spmd(nc, in_maps, core_ids=list(range(NCORES)))
    out = np.concatenate([r["out"] for r in res.results], axis=0)
    wts = np.concatenate([r["wts"] for r in res.results], axis=0)
    return out, wts


# revision 19
# speedup vs baseline: 1.9958x; 1.9958x over previous
"""Bahdanau 1D attention Trainium2 kernel.

Math (mask is all-ones per the problem spec, so it is algebraically dropped):
  values[t,u] = x[t,:] @ W            (computed transposed: [U, T] on chip)
  score[t]    = sum_u v_u * tanh(b_u + values[t,u])
  e[t]        = exp(score[t]);  S = sum_t e[t]
  weights[t]  = e[t] / S
  output[u]   = sum_t weights[t] * values[t,u]
              = ((sum_t e[t] * x[t,:]) @ W) / S        <- linear factorization:
  the weighted sum commutes with the matmul, so raw `values` in [T,U] layout
  are never needed - only tanh(values^T) for scores.

Sharding: data-parallel over batch, 8 batches per core on 8 cores.

Per-core loop (8 batches x 16 chunks of T=512):
  - DMA x in 2 MiB pieces  [128(T%128), 4096] natural layout
  - PE transposes x -> xT [D, T] (PSUM), DVE copies PSUM->SBUF
  - PE values^T = W^T @ xT (fp32r, full rate at N=512)
  - ACT tanh with per-partition bias b
  - PE score^T columns [128,1] via y-stationary matmuls against v
  - ACT exp -> e_all [128, 64] per batch
  - PE xw += e^T @ x   (accumulates [1, 256] in PSUM across the whole batch)
  - batch epilogue: S, 1/S, weights = e/S (transposed out), out = (xw@W)/S
"""

import sys

for _p in ("/opt/trn_rl_repo",):
    if _p not in sys.path:
        sys.path.insert(0, _p)

import numpy as np

B, T, D, U = 64, 8192, 256, 256
NCORES = 8
BPC = B // NCORES          # batches per core
PIECE = 2048               # T rows per DMA
CHUNK = 512                # T rows per compute chunk
NCHUNK = T // CHUNK        # 16
NPIECE = T // PIECE        # 4
SUBPC = PIECE // CHUNK     # 4

_CACHE = {}


def _build_program():
    import concourse.bass as bass  # noqa: F401
    import concourse.tile as tile
    from concourse import bacc, mybir
    from contextlib import ExitStack

    f32 = mybir.dt.float32
    f32r = mybir.dt.float32r
    bf16 = mybir.dt.bfloat16  # noqa: F841
    fp16 = mybir.dt.float16
    AF = mybir.ActivationFunctionType

    nc = bacc.Bacc(
        "TRN2", target_bir_lowering=False, debug=False, num_devices=NCORES
    )

    x_d = nc.dram_tensor("x", [BPC, T, D], fp16, kind="ExternalInput").ap()
    w_d = nc.dram_tensor("wmat", [D, U], f32r, kind="ExternalInput").ap()
    b_d = nc.dram_tensor("bvec", [128, 2], f32, kind="ExternalInput").ap()
    v_d = nc.dram_tensor("vvec", [128, 4], f32r, kind="ExternalInput").ap()
    id_d = nc.dram_tensor("ident", [128, 128], f32r, kind="ExternalInput").ap()
    onc_d = nc.dram_tensor("onesc", [128, 2], f32r, kind="ExternalInput").ap()
    onr_d = nc.dram_tensor("onesr", [1, 128], f32r, kind="ExternalInput").ap()
    out_d = nc.dram_tensor("out", [BPC, U], f32, kind="ExternalOutput").ap()
    wts_d = nc.dram_tensor("wts", [BPC, T], f32, kind="ExternalOutput").ap()

    def r(ap):  # fp32 -> fp32r view for full-rate PE matmuls
        return ap.bitcast(f32r)

    with tile.TileContext(nc) as tc, ExitStack() as ctx:
        const = ctx.enter_context(tc.tile_pool(name="const", bufs=1))
        xin = ctx.enter_context(tc.tile_pool(name="xin", bufs=3))
        xt_sb = ctx.enter_context(tc.tile_pool(name="xtsb", bufs=6))
        xt_ps = ctx.enter_context(tc.tile_pool(name="xtps", bufs=2, space="PSUM"))
        v_ps = ctx.enter_context(tc.tile_pool(name="vps", bufs=2, space="PSUM"))
        y_sb = ctx.enter_context(tc.tile_pool(name="ysb", bufs=6))
        s_ps = ctx.enter_context(tc.tile_pool(name="sps", bufs=1, space="PSUM"))
        e_sb = ctx.enter_context(tc.tile_pool(name="esb", bufs=2))
        e16_sb = ctx.enter_context(tc.tile_pool(name="e16sb", bufs=3))
        xw_ps = ctx.enter_context(tc.tile_pool(name="xwps", bufs=2, space="PSUM"))
        epi_ps = ctx.enter_context(tc.tile_pool(name="epips", bufs=1, space="PSUM"))
        epi_sb = ctx.enter_context(tc.tile_pool(name="episb", bufs=2))

        W_sb = const.tile([128, 512], f32r, tag="W")  # col h*256+u = W[h*128+d, u]
        nc.sync.dma_start(W_sb[:, 0:256], w_d[0:128, :])
        nc.sync.dma_start(W_sb[:, 256:512], w_d[128:256, :])
        b_sb = const.tile([128, 2], f32, tag="bv")
        nc.sync.dma_start(b_sb[:, :], b_d[:, :])
        v_sb = const.tile([128, 4], f32r, tag="vv")
        nc.sync.dma_start(v_sb[:, :], v_d[:, :])
        ident = const.tile([128, 128], f32r, tag="id")
        nc.sync.dma_start(ident[:, :], id_d[:, :])
        onesc = const.tile([128, 2], f32r, tag="onc")
        nc.sync.dma_start(onesc[:, :], onc_d[:, :])
        onesr = const.tile([1, 128], f32r, tag="onr")
        nc.sync.dma_start(onesr[:, :], onr_d[:, :])
        v_bf = const.tile([128, 4], fp16, tag="vbf")
        nc.vector.tensor_copy(v_bf[:, :], v_sb[:, :].bitcast(f32))
        W_h16 = const.tile([128, 512], fp16, tag="Wh")
        nc.vector.tensor_copy(W_h16[:, :], W_sb[:, :].bitcast(f32))
        ident_h = const.tile([128, 128], fp16, tag="idh")
        nc.vector.tensor_copy(ident_h[:, :], ident[:, :].bitcast(f32))

        batch_state = {}

        def emit_back(st):
            """score -> exp -> xw for a completed front-stage chunk."""
            bi, c, sc, xbig, ys = st
            if c == 0:
                e_all = e_sb.tile([128, NCHUNK * 4], f32r, tag="e",
                                  name=f"eall{bi}")
                xw = xw_ps.tile([1, 256], f32, tag="xw", name=f"xw{bi}")
                batch_state[bi] = (e_all, xw)
            e_all, xw = batch_state[bi]
            s = s_ps.tile([128, 8], f32, tag="s", name=f"s{bi}_{c}")
            for g in range(4):
                for m in range(2):
                    nc.tensor.matmul(
                        s[:, 2 * g:2 * g + 2],
                        ys[m][:, g * 128:(g + 1) * 128],
                        v_bf[:, 2 * m:2 * m + 2],
                        start=(m == 0), stop=(m == 1),
                        skip_group_check=True)
            s_even = s[:, :].rearrange("p (g two) -> p g two", two=2)[:, :, 0:1]
            e_dst = e_all[:, c * 4:(c + 1) * 4].rearrange(
                "p (g one) -> p g one", one=1)
            nc.scalar.activation(e_dst, s_even, AF.Exp)
            e16 = e16_sb.tile([128, 4], fp16, tag="e16", name=f"e16_{bi}_{c}")
            nc.scalar.activation(
                e16[:, :], e_all[:, c * 4:(c + 1) * 4].bitcast(f32), AF.Copy)
            for g in range(4):
                nc.tensor.matmul(
                    xw[:, :],
                    e16[:, g:g + 1],
                    xbig[:, (sc * 4 + g) * 256:(sc * 4 + g + 1) * 256],
                    start=(c == 0 and g == 0),
                    stop=(c == NCHUNK - 1 and g == 3),
                    skip_group_check=True)

        def emit_epilogue(bi):
            e_all, xw = batch_state.pop(bi)
            sum_col = epi_sb.tile([128, 1], f32, tag="sumcol",
                                  name=f"sumcol{bi}")
            nc.vector.tensor_reduce(
                sum_col[:, :], e_all[:, :],
                axis=mybir.AxisListType.X, op=mybir.AluOpType.add)
            sum_col_r = epi_sb.tile([128, 1], f32r, tag="sumcolr",
                                    name=f"sumcolr{bi}")
            nc.vector.tensor_copy(sum_col_r[:, :], sum_col[:, :])
            stot_ps = epi_ps.tile([1, 2], f32, tag="epiA", name=f"stot{bi}")
            nc.tensor.matmul(stot_ps[:, :], sum_col_r[:, :], onesc[:, :],
                             start=True, stop=True, skip_group_check=True)
            rcp = epi_sb.tile([1, 2], f32, tag="rcp", name=f"rcp{bi}")
            nc.vector.reciprocal(rcp[:, :], stot_ps[0:1, 0:2])
            rcp_r = epi_sb.tile([1, 2], f32r, tag="rcpr", name=f"rcpr{bi}")
            nc.vector.tensor_copy(rcp_r[:, :], rcp[:, :])
            rep_ps = epi_ps.tile([128, 2], f32, tag="epiA", name=f"rep{bi}")
            nc.tensor.matmul(rep_ps[:, :], onesr[:, :], rcp_r[:, :],
                             start=True, stop=True, skip_group_check=True)
            rcol = epi_sb.tile([128, 1], f32, tag="rcol", name=f"rcol{bi}")
            nc.vector.tensor_copy(rcol[:, :], rep_ps[:, 0:1])
            eT_ps = epi_ps.tile([64, 128], f32, tag="epiA", name=f"eT{bi}")
            nc.tensor.matmul(r(eT_ps[:, :]), r(e_all[:, :]), r(ident[:, :]),
                             is_transpose=True, start=True, stop=True,
                             skip_group_check=True)
            wT_sb = epi_sb.tile([64, 128], f32, tag="wT", name=f"wT{bi}")
            nc.scalar.activation(wT_sb[:, :], eT_ps[:, :], AF.Copy,
                                 scale=rcol[0:64, :])
            nc.sync.dma_start(
                wts_d[bi].rearrange("(c q) -> c q", q=128), wT_sb[:, :])
            xw_sb = epi_sb.tile([1, 256], f32r, tag="xwsb", name=f"xwsb{bi}")
            nc.vector.tensor_copy(xw_sb[:, :], xw[:, :])
            xwT_ps = epi_ps.tile([128, 4], f32, tag="epiA", name=f"xwT{bi}")
            for h in range(2):
                nc.tensor.matmul(
                    r(xwT_ps[:, 2 * h:2 * h + 2]),
                    r(xw_sb[:, h * 128:(h + 1) * 128]),
                    onesr[0:1, 0:2], is_transpose=True,
                    start=True, stop=True, skip_group_check=True)
            xwT_sb = epi_sb.tile([128, 4], f32r, tag="xwT", name=f"xwTs{bi}")
            nc.vector.tensor_copy(xwT_sb[:, :], r(xwT_ps[:, :]))
            out_ps = epi_ps.tile([1, 256], f32, tag="epiA", name=f"outps{bi}")
            for h in range(2):
                nc.tensor.matmul(out_ps[:, :], r(xwT_sb[:, 2 * h:2 * h + 1]),
                                 r(W_sb[:, h * 256:(h + 1) * 256]),
                                 start=(h == 0), stop=(h == 1),
                                 skip_group_check=True)
            out_sb = epi_sb.tile([1, 256], f32, tag="osb", name=f"osb{bi}")
            nc.scalar.activation(out_sb[:, :], out_ps[:, :], AF.Copy,
                                 scale=rcp[0:1, 0:1])
            nc.sync.dma_start(out_d[bi:bi + 1, :], out_sb[:, :])

        pend = None
        for bi in range(BPC):
            for piece in range(NPIECE):
                xbig = xin.tile([128, PIECE * D // 128], fp16, tag="x",
                                name=f"xbig{bi}_{piece}")
                src = x_d[bi, piece * PIECE:(piece + 1) * PIECE, :].rearrange(
                    "(k p) d -> p k d", p=128
                )
                dst = xbig[:, :].rearrange("p (k d) -> p k d", d=D)
                nc.sync.dma_start(dst, src)

                for sc in range(SUBPC):
                    c = piece * SUBPC + sc
                    xt0 = xt_ps.tile([128, 512], fp16, tag="xt",
                                     name=f"xt0_{bi}_{c}")
                    xt1 = xt_ps.tile([128, 512], fp16, tag="xt",
                                     name=f"xt1_{bi}_{c}")
                    for g in range(4):
                        xv = xbig[:, (sc * 4 + g) * 256:(sc * 4 + g + 1) * 256]
                        nc.tensor.matmul(
                            xt0[:, g * 128:(g + 1) * 128], xv[:, 0:128],
                            ident_h[:, :], is_transpose=True,
                            start=True, stop=True, skip_group_check=True)
                        nc.tensor.matmul(
                            xt1[:, g * 128:(g + 1) * 128], xv[:, 128:256],
                            ident_h[:, :], is_transpose=True,
                            start=True, stop=True, skip_group_check=True)
                    xts = [xt_sb.tile([128, 512], fp16, tag="xts",
                                      name=f"xts{_h}") for _h in range(2)]
                    nc.vector.tensor_copy(xts[0][:, :], xt0[:, :])
                    nc.vector.tensor_copy(xts[1][:, :], xt1[:, :])
                    vps = [v_ps.tile([128, 512], f32, tag="v",
                                     name=f"vps{_h}") for _h in range(2)]
                    for m in range(2):
                        for h in range(2):
                            nc.tensor.matmul(
                                vps[m][:, :],
                                W_h16[:, h * 256 + m * 128:h * 256 + (m + 1) * 128],
                                xts[h][:, :],
                                start=(h == 0), stop=(h == 1),
                                skip_group_check=True)
                    ys = [y_sb.tile([128, 512], fp16, tag="y",
                                    name=f"ys{_h}") for _h in range(2)]
                    for m in range(2):
                        nc.scalar.activation(
                            ys[m][:, :], vps[m][:, :], AF.Tanh,
                            bias=b_sb[:, m:m + 1], scale=1.0)
                    if pend is not None:
                        emit_back(pend)
                        if pend[1] == NCHUNK - 1:
                            emit_epilogue(pend[0])
                    pend = (bi, c, sc, xbig, ys)
        emit_back(pend)
        emit_epilogue(pend[0])

    nc.compile()
    return nc


def _get_program():
    if "nc" not in _CACHE:
        _CACHE["nc"] = _build_program()
    return _CACHE["nc"]


def _make_in_maps(x, W, b, v):
    b2 = np.ascontiguousarray(b.reshape(2, 128).T)
    v2c = v.reshape(2, 128).T
    v4 = np.empty((128, 4), dtype=np.float32)
    v4[:, 0] = v2c[:, 0]
    v4[:, 1] = v2c[:, 0]
    v4[:, 2] = v2c[:, 1]
    v4[:, 3] = v2c[:, 1]
    eye = np.eye(128, dtype=np.float32)
    onesc = np.ones((128, 2), dtype=np.float32)
    onesr = np.ones((1, 128), dtype=np.float32)
    x16 = np.asarray(x, dtype=np.float16)
    return [{
        "x": np.ascontiguousarray(x16[c * BPC:(c + 1) * BPC]),
        "wmat": W, "bvec": b2, "vvec": v4,
        "ident": eye, "onesc": onesc, "onesr": onesr,
    } for c in range(NCORES)]


def kernel(**inputs):
    from concourse.bass_utils import run_bass_kernel_spmd

    x = np.ascontiguousarray(np.asarray(inputs["inputs"], dtype=np.float32))
    W = np.ascontiguousarray(np.asarray(inputs["W"], dtype=np.float32))
    b = np.asarray(inputs["b"], dtype=np.float32)
    v = np.asarray(inputs["v"], dtype=np.float32)
    # mask is all-ones by problem construction; algebraically a no-op.

    nc = _get_program()
    in_maps = _make_in_maps(x, W, b, v)
    res = run_bass_kernel_spmd(nc, in_maps, core_ids=list(range(NCORES)))
    out = np.concatenate([r["out"] for r in res.results], axis=0)
    wts = np.concatenate([r["wts"] for r in res.results], axis=0)
    return out, wts
